# revision 1
# baseline (speedup 1.0000x reference)
"""Trainium2 Bass kernel for nn_AttentionBasisSynthesizer.

out[b] = softmax(Q[b] @ K[b].T + bias) @ V[b], bias[k] built from a tiny
sinusoidal atom bank (computed on host, replicated to every core).

Sharding: data-parallel over the batch dim — 8 batches onto 8 NeuronCores,
one batch per core. Each core computes its full [S, S] attention.

Device-side layout (per core): scores are computed TRANSPOSED, sT[k, q], so
- the key-dim bias is per-partition and folds into the ACT engine's free
  exp(scale*x + bias) affine,
- P @ V needs no transposes: out.T[d, q] = sum over k-tiles of
  matmul(lhsT=V_tile[k,d], rhs=exp_scores[k, q]).
Softmax uses a constant shift C instead of a per-row max (mathematically
exact; scores ~ N(0, sqrt(128)) so exp(s - C) can neither overflow nor
flush the row maximum for any plausible data).
The denominator Z[q] = sum_k p[k, q] is a partition-axis sum: p-tiles are
pairwise-folded in bf16 on the Vector engine (cheap 2x mode), and the last
128-partition sum is a ones-matmul on the Tensor engine into PSUM.
"""

import numpy as np

import concourse.bass as bass
import concourse.tile as tile
from concourse import mybir
from concourse.bass_utils import run_bass_kernel_spmd
from concourse.vector_clock import ScopedClock

B, S, D = 8, 2048, 128
KT = S // 128          # 16 key tiles of 128
NCH = S // 512         # 4 psum chunks of 512
C_SHIFT = 64.0         # constant softmax shift (exact: softmax(x-C)=softmax(x))

F32 = mybir.dt.float32
F32R = mybir.dt.float32r
BF16 = mybir.dt.bfloat16
EXP = mybir.ActivationFunctionType.Exp


def _install_tile_drain_patch():
    """This container's walrus accepts only one semaphore wait per sync-queue
    instruction, but TileContext's tail drain carries one wait per tracked
    proc. Split the waits across single-wait NOPs ahead of the drain (the
    sync queue is in-order, so the drain still begins only after every wait
    has been satisfied)."""

    def _drain_and_barrier(self, tick_clock, wait_clock):
        nc = self.nc
        probe = nc.sync.nop()
        wait_clock.add_sem_waits(
            probe.ins, ScopedClock({None: tick_clock.global_clock})
        )
        si = probe.ins.sync_info
        waits = list(si.on_wait or []) if si is not None else []
        if len(waits) > 1:
            si.on_wait = waits[:1]
            # distribute the remaining waits across all engine queues so the
            # checks evaluate in parallel; the following all-engine barrier
            # joins them back together.
            engines = [nc.sync, nc.scalar, nc.vector, nc.gpsimd, nc.tensor]
            for i, w in enumerate(waits[1:]):
                extra = engines[i % len(engines)].nop()
                extra.ins.sync_info = mybir.SyncInfo(on_wait=[w], on_update=[])
        nc.sync.drain()
        nc.all_engine_barrier()
        assert self.sems is not None
        popped = nc._tile_sem_poison_stack.pop()
        assert popped is self._sem_poison
        nc.clear_and_free_semaphores(list(self.sems.allocated().values()))
        nc.all_engine_barrier()

    tile.TileContext._drain_and_barrier = _drain_and_barrier


def _split_multi_waits(nc: bass.Bass, limit: int = 1) -> int:
    """This container's walrus rejects instructions carrying more than one
    semaphore wait ("Too many sync wait commands"). Hoist excess waits onto
    same-engine NOPs inserted immediately before the instruction — engine
    queues dispatch in order, so the semantics are identical."""
    n_split = 0
    for fn in nc.m.functions:
        for blk in fn.blocks:
            insts = blk.instructions
            out = []
            for inst in insts:
                si = inst.sync_info
                waits = list(si.on_wait or []) if si is not None else []
                if len(waits) > limit:
                    keep = waits[:limit]
                    extra = waits[limit:]
                    for j in range(0, len(extra), limit):
                        nop = mybir.InstNoOp(
                            name=f"{inst.name}-waitsplit{j}",
                            ins=[],
                            outs=[],
                            engine=inst.engine,
                        )
                        nop.sync_info = mybir.SyncInfo(
                            on_wait=extra[j : j + limit], on_update=[]
                        )
                        nc.register_instruction(nop, overwrite=True)
                        out.append(nop)
                        n_split += 1
                    si.on_wait = keep
                out.append(inst)
            if n_split:
                blk.instructions = out
    return n_split


def build_nc(reps: int = 1) -> bass.Bass:
    """reps>1 unrolls the whole body (including input loads) that many times
    inside one NEFF — used only by the test harness to measure steady-state
    per-execution HW time as a marginal; the graded path uses reps=1."""
    _install_tile_drain_patch()
    nc = bass.Bass()

    qT = nc.declare_dram_parameter("qT", [D, S], F32R, isOutput=False)
    kT = nc.declare_dram_parameter("kT", [D, S], F32R, isOutput=False)
    v = nc.declare_dram_parameter("v", [S, D], F32, isOutput=False)
    biasb = nc.declare_dram_parameter("biasb", [128, KT], F32, isOutput=False)
    oT = nc.declare_dram_parameter("oT", [D, S], F32, isOutput=True)

    with tile.TileContext(nc) as tc:
        with (
            tc.tile_pool(name="const", bufs=1) as const,
            tc.tile_pool(name="pp", bufs=5) as pp,
            tc.tile_pool(name="l1p", bufs=3) as l1p,
            tc.tile_pool(name="l2p", bufs=3) as l2p,
            tc.tile_pool(name="l3p", bufs=3) as l3p,
            tc.tile_pool(name="l4p", bufs=2) as l4p,
            tc.tile_pool(name="outp", bufs=1) as outp,
            tc.tile_pool(name="sps", bufs=2, space="PSUM") as sps,
            tc.tile_pool(name="ops", bufs=1, space="PSUM") as ops,
        ):
            def _emit_body():
                kTs = const.tile([D, S], F32R)
                qTs = const.tile([D, S], F32R)
                bias_s = const.tile([128, KT], F32)
                ones_s = const.tile([128, 128], BF16)
                vstage = const.tile([128, KT, D], F32)
                vb = const.tile([128, KT, D], BF16)

                # DMA order matters: queue issue costs ~0.6us per dma_start, so
                # few, large DMAs — split only where the pipeline needs early
                # availability (kT tile 0 + the first qT half feed QK(0)). The
                # v/bias loads ride the SWDGE queue in parallel.
                nc.gpsimd.dma_start(bias_s[:], biasb[:])
                nc.vector.memset(ones_s[:], 1.0)
                # PE warmup: the HAM clock gate holds the PE at 1.2 GHz until
                # it has been busy ~3.4us. The PE is otherwise idle during the
                # input DMA lead-in, so trip the gate now with throwaway
                # matmuls on a zeroed tile (result never read; the psum slot
                # recycles into the QK rotation afterwards).
                warm_w = const.tile([128, 128], BF16, tag="warm_w")
                nc.vector.memset(warm_w[:], 0.0)
                warm_ps = sps.tile([128, 1024], F32, tag="sp", name="warm_ps")
                for _w in range(48):
                    nc.tensor.matmul(
                        warm_ps[:, 0:128], lhsT=warm_w[:], rhs=warm_w[:],
                        start=True, stop=True,
                    )
                nc.sync.dma_start(kTs[:, 0:128], kT[:, 0:128])
                nc.sync.dma_start(qTs[:, 0:512], qT[:, 0:512])
                nc.sync.dma_start(qTs[:, 512:1024], qT[:, 512:1024])
                nc.sync.dma_start(kTs[:, 128:512], kT[:, 128:512])
                nc.sync.dma_start(qTs[:, 1024:2048], qT[:, 1024:2048])
                nc.sync.dma_start(kTs[:, 512:2048], kT[:, 512:2048])
                for g in range(NCH):
                    ki0 = g * 4
                    src = v[ki0 * 128 : (ki0 + 4) * 128, :].rearrange(
                        "(t p) d -> p t d", p=128
                    )
                    nc.gpsimd.dma_start(vstage[:, ki0 : ki0 + 4, :], src)
                    nc.vector.tensor_copy(vb[:, ki0 : ki0 + 4, :],
                                          vstage[:, ki0 : ki0 + 4, :])

                o_ps = ops.tile([128, S], F32, tag="o")

                # Z fold tree operates on [128, 1024] halves so the tail levels
                # pipeline behind the exp halves instead of serializing after the
                # last full tile. The tree is imbalanced: p0..p7 and p8..p13 are
                # pre-folded during the loop, so only two add-levels remain after
                # the final exp ((p14+p15), then acc).
                HV = 2  # halves
                p_tiles = []
                pools = {1: (l1p, "l1"), 2: (l2p, "l2"), 3: (l3p, "l3"), 4: (l4p, "l4")}

                def fold(level, parent_pair, name):
                    pool, tag = pools[level]
                    t = pool.tile([128, S], BF16, tag=tag, name=name)
                    a, b_ = parent_pair
                    for h in range(HV):
                        sl = slice(h * (S // HV), (h + 1) * (S // HV))
                        nc.vector.tensor_add(t[:, sl], a[:, sl], b_[:, sl])
                    return t

                folds = {}

                for ki in range(KT):
                    p_t = pp.tile([128, S], BF16, tag="p")
                    for h in range(2):
                        sp = sps.tile([128, 1024], F32, tag="sp")
                        for c in range(2):
                            q0 = c * 512
                            nc.tensor.matmul(
                                sp[:, q0 : q0 + 512],
                                lhsT=kTs[:, ki * 128 : (ki + 1) * 128],
                                rhs=qTs[:, h * 1024 + q0 : h * 1024 + q0 + 512],
                                start=True,
                                stop=True,
                            )
                        nc.scalar.activation(
                            p_t[:, h * 1024 : (h + 1) * 1024],
                            sp[:],
                            EXP,
                            bias=bias_s[:, ki : ki + 1],
                            scale=1.0,
                        )
                    # PV accumulation into oT psum
                    for c in range(NCH):
                        nc.tensor.matmul(
                            o_ps[:, c * 512 : (c + 1) * 512],
                            lhsT=vb[:, ki, :],
                            rhs=p_t[:, c * 512 : (c + 1) * 512],
                            start=(ki == 0),
                            stop=(ki == KT - 1),
                        )
                    # Z folding tree (bf16, DVE 2x mode), imbalanced for a short
                    # tail: each entry is (result_range, left_range, right_range);
                    # a range of a single index means the raw p tile.
                    p_tiles.append(p_t)
                    plan = {
                        1: [((0, 1), (0, 0), (1, 1))],
                        3: [((2, 3), (2, 2), (3, 3)), ((0, 3), (0, 1), (2, 3))],
                        5: [((4, 5), (4, 4), (5, 5))],
                        7: [((6, 7), (6, 6), (7, 7)), ((4, 7), (4, 5), (6, 7)),
                            ((0, 7), (0, 3), (4, 7))],
                        9: [((8, 9), (8, 8), (9, 9))],
                        11: [((10, 11), (10, 10), (11, 11)),
                             ((8, 11), (8, 9), (10, 11))],
                        13: [((12, 13), (12, 12), (13, 13)),
                             ((8, 13), (8, 11), (12, 13)),
                             ((0, 13), (0, 7), (8, 13))],
                        15: [((14, 15), (14, 14), (15, 15)),
                             ((0, 15), (0, 13), (14, 15))],
                    }

                    def get(rng):
                        return p_tiles[rng[0]] if rng[0] == rng[1] else folds[rng]

                    for rng, left, right in plan.get(ki, []):
                        size = rng[1] - rng[0] + 1
                        lvl = {2: 1, 4: 2, 6: 3, 8: 3, 14: 4, 16: 4}[size]
                        folds[rng] = fold(
                            lvl, (get(left), get(right)), f"f{rng[0]}_{rng[1]}"
                        )

                acc = folds[(0, 15)]

                # Tail: Z ones-matmuls (N<=512 per PSUM bank), then per-1024-half
                # reciprocal -> normalize -> store, interleaved for overlap.
                rz = outp.tile([128, S], F32, tag="rz")
                oTs = outp.tile([128, S], F32, tag="oTs")
                zts = [
                    sps.tile([128, 1024], F32, tag="sp", name=f"zt{i}")
                    for i in range(2)
                ]
                prev_mul = None
                for h in range(2):
                    zt = zts[h]
                    for c in range(2):
                        nc.tensor.matmul(
                            zt[:, c * 512 : (c + 1) * 512],
                            lhsT=ones_s[:],
                            rhs=acc[:, h * 1024 + c * 512 : h * 1024 + (c + 1) * 512],
                            start=True,
                            stop=True,
                        )
                    sl = slice(h * 1024, (h + 1) * 1024)
                    rec_i = nc.vector.reciprocal(rz[:, sl], zt[:])
                    if prev_mul is not None:
                        # keep DVE in recip->mul->recip->mul order so the first
                        # output store starts as early as possible
                        tile.add_dep_helper(
                            rec_i.ins, prev_mul.ins, sync=False,
                            reason="tail h-order",
                        )
                    prev_mul = nc.vector.tensor_mul(oTs[:, sl], o_ps[:, sl], rz[:, sl])
                    nc.sync.dma_start(oT[:, sl], oTs[:, sl])

            for _rep in range(reps):
                _emit_body()

    _split_multi_waits(nc)
    return nc


def _bias_kernel(waveforms, gains, window, atom_indices, shifts) -> np.ndarray:
    waveforms = np.asarray(waveforms, dtype=np.float32)
    gains = np.asarray(gains, dtype=np.float32)
    window = np.asarray(window, dtype=np.float32)
    atom_indices = np.asarray(atom_indices).astype(np.int64)
    shifts = np.asarray(shifts).astype(np.int64)
    atoms = waveforms[atom_indices, :S]                  # [P, S]
    bases = atoms * gains[:, None]                       # [P, S]
    shifted = np.stack(
        [np.roll(bases[p], shifts[p]) for p in range(bases.shape[0])]
    )
    return (shifted * window[None, :S]).sum(0).astype(np.float32)  # [S]


def kernel(queries, keys, values, waveforms, gains, window, atom_indices, shifts):
    queries = np.asarray(queries, dtype=np.float32)
    keys = np.asarray(keys, dtype=np.float32)
    values = np.asarray(values, dtype=np.float32)

    bias = _bias_kernel(waveforms, gains, window, atom_indices, shifts)
    biasb = np.ascontiguousarray((bias - C_SHIFT).reshape(KT, 128).T)  # [128, KT]

    nc = build_nc()
    in_maps = [
        {
            "qT": np.ascontiguousarray(queries[b].T),
            "kT": np.ascontiguousarray(keys[b].T),
            "v": np.ascontiguousarray(values[b]),
            "biasb": biasb,
        }
        for b in range(B)
    ]
    res = run_bass_kernel_spmd(nc, in_maps, list(range(B)))
    out = np.stack([np.ascontiguousarray(res.results[b]["oT"].T) for b in range(B)])
    return out.astype(np.float32)



# revision 26
# speedup vs baseline: 1.1649x; 1.1649x over previous
"""Trainium2 Bass kernel for nn_AttentionBasisSynthesizer.

out[b] = softmax(Q[b] @ K[b].T + bias) @ V[b], bias[k] built from a tiny
sinusoidal atom bank (computed on host, replicated to every core).

Sharding: data-parallel over the batch dim — 8 batches onto 8 NeuronCores,
one batch per core. Each core computes its full [S, S] attention.

Device-side layout (per core): scores are computed TRANSPOSED, sT[k, q], so
- the key-dim bias is per-partition and folds into the ACT engine's free
  exp(scale*x + bias) affine,
- P @ V needs no transposes: out.T[d, q] = sum over k-tiles of
  matmul(lhsT=V_tile[k,d], rhs=exp_scores[k, q]).
Softmax uses a constant shift C instead of a per-row max (mathematically
exact; scores ~ N(0, sqrt(128)) so exp(s - C) can neither overflow bf16 nor
flush the row maximum for any plausible data).

v2 structure (engine balance + head/tail overlap):
- The q axis is processed in two halves of 1024. PSUM: one persistent
  [128, 2048] o_ps accumulator (4 banks) + two [128, 1024] score tiles
  (2 banks each) in rotation = exactly 8 banks.
- Per (half, k-tile): QK (2x N=512 fp32r matmuls) -> exp (one [128,1024]
  ACT instr, bias via the per-partition bias port) -> PV (2x N=512 bf16
  matmuls accumulating o_ps). Emission order keeps QK one tile ahead so
  the ACT engine (the busiest) never waits.
- Z[q] = sum_k p[k,q] is a linear bf16 fold chain on the DVE over p-tiles
  0..14; the final 128-partition reduction is ones^T @ acc + ones^T @ p15,
  PSUM-accumulated into the OPPOSITE half's o_ps region (idle banks), so
  no fold work remains after the last exp.
- Half A's tail (reciprocal, normalize, store) overlaps half B's main
  loop; half B's last tile is chunked 2x512 so the closing pipeline
  (Z-matmul -> recip -> mul -> DMA) is short.
- A few exp tiles are offloaded from ACT to the DVE via the Schraudolph
  bit-trick: bf16(e^x) ~= bitcast_bf16(uint16(A*x + B)), one tensor_scalar
  (mult-imm, add-per-partition-vec) per tile; the f32->uint16 saturating
  convert clamps underflow to +0.0. This balances ACT vs DVE busy time.
- V is converted to bf16 and laid out [k%128, k//128, d] on the host.
"""

import numpy as np

import concourse.bass as bass
import concourse.tile as tile
from concourse import mybir
from concourse.bass_utils import run_bass_kernel_spmd
from concourse.vector_clock import ScopedClock

B, S, D = 8, 2048, 128
KT = S // 128           # 16 key tiles of 128
HW = S // 2             # half width (q) = 1024
C_SHIFT = 20.0          # constant softmax shift (exact: softmax(x-C)=softmax(x))

# Schraudolph exp constants for bf16 (unit in last place of exponent = 128):
# e^x ~= bitcast_bf16(uint16(A*x + B)); B = 127*128 - C0, C0 tuned minimax.
SCH_A = 128.0 / float(np.log(2.0))
SCH_B0 = 128.0 * 127.0 - 5.5

F32 = mybir.dt.float32
F32R = mybir.dt.float32r
BF16 = mybir.dt.bfloat16
U16 = mybir.dt.uint16
EXP = mybir.ActivationFunctionType.Exp
MULT = mybir.AluOpType.mult
ADD = mybir.AluOpType.add

# (half, ki) tiles whose exp runs on the DVE via the bit-trick instead of ACT
DVE_OFF = ()
WARM = 1                # PE warmup matmul count
FZ = 1                  # trailing p-tiles per half reduced via Z-matmul
                        # instead of the DVE fold chain


def _install_tile_drain_patch():
    """This container's walrus accepts only one semaphore wait per sync-queue
    instruction, but TileContext's tail drain carries one wait per tracked
    proc. Split the waits across single-wait NOPs ahead of the drain (the
    sync queue is in-order, so the drain still begins only after every wait
    has been satisfied)."""

    def _drain_and_barrier(self, tick_clock, wait_clock):
        nc = self.nc
        probe = nc.sync.nop()
        wait_clock.add_sem_waits(
            probe.ins, ScopedClock({None: tick_clock.global_clock})
        )
        si = probe.ins.sync_info
        waits = list(si.on_wait or []) if si is not None else []
        if len(waits) > 1:
            si.on_wait = waits[:1]
            # distribute the remaining waits across all engine queues so the
            # checks evaluate in parallel; the following all-engine barrier
            # joins them back together.
            engines = [nc.sync, nc.scalar, nc.vector, nc.gpsimd, nc.tensor]
            for i, w in enumerate(waits[1:]):
                extra = engines[i % len(engines)].nop()
                extra.ins.sync_info = mybir.SyncInfo(on_wait=[w], on_update=[])
        nc.sync.drain()
        nc.all_engine_barrier()
        assert self.sems is not None
        popped = nc._tile_sem_poison_stack.pop()
        assert popped is self._sem_poison
        nc.clear_and_free_semaphores(list(self.sems.allocated().values()))
        nc.all_engine_barrier()

    tile.TileContext._drain_and_barrier = _drain_and_barrier


def _split_multi_waits(nc: bass.Bass, limit: int = 1) -> int:
    """This container's walrus rejects instructions carrying more than one
    semaphore wait ("Too many sync wait commands"). Hoist excess waits onto
    same-engine NOPs inserted immediately before the instruction — engine
    queues dispatch in order, so the semantics are identical."""
    n_split = 0
    for fn in nc.m.functions:
        for blk in fn.blocks:
            insts = blk.instructions
            out = []
            for inst in insts:
                si = inst.sync_info
                waits = list(si.on_wait or []) if si is not None else []
                if len(waits) > limit:
                    keep = waits[:limit]
                    extra = waits[limit:]
                    for j in range(0, len(extra), limit):
                        nop = mybir.InstNoOp(
                            name=f"{inst.name}-waitsplit{j}",
                            ins=[],
                            outs=[],
                            engine=inst.engine,
                        )
                        nop.sync_info = mybir.SyncInfo(
                            on_wait=extra[j : j + limit], on_update=[]
                        )
                        nc.register_instruction(nop, overwrite=True)
                        out.append(nop)
                        n_split += 1
                    si.on_wait = keep
                out.append(inst)
            if n_split:
                blk.instructions = out
    return n_split


def build_nc(reps: int = 1, warm: int = WARM, dve_off=DVE_OFF,
             fz: int = FZ) -> bass.Bass:
    """reps>1 unrolls the whole body (including input loads) that many times
    inside one NEFF — used only by the test harness to measure steady-state
    per-execution HW time as a marginal; the graded path uses reps=1."""
    _install_tile_drain_patch()
    dve_off = frozenset(dve_off)
    nc = bass.Bass()

    qT = nc.declare_dram_parameter("qT", [D, S], F32R, isOutput=False)
    kT = nc.declare_dram_parameter("kT", [D, S], F32R, isOutput=False)
    vb = nc.declare_dram_parameter("vb", [128, KT * D], BF16, isOutput=False)
    biasb = nc.declare_dram_parameter("biasb", [128, KT], F32, isOutput=False)
    sbias = nc.declare_dram_parameter("sbias", [128, KT], F32, isOutput=False)
    oT = nc.declare_dram_parameter("oT", [D, S], F32, isOutput=True)

    with tile.TileContext(nc) as tc:
        with (
            tc.tile_pool(name="const", bufs=1) as const,
            tc.tile_pool(name="pp", bufs=4) as pp,
            tc.tile_pool(name="sps", bufs=2, space="PSUM") as sps,
            tc.tile_pool(name="ops", bufs=1, space="PSUM") as ops,
        ):
            accp = tails = pp
            def _emit_body():
                kTs = const.tile([D, S], F32R, tag="kTs")
                qTs = const.tile([D, S], F32R, tag="qTs")
                bias_s = const.tile([128, KT], F32, tag="bias")
                sbias_s = const.tile([128, KT], F32, tag="sbias")
                ones_s = const.tile([128, 128], BF16, tag="ones")
                vbs = const.tile([128, KT, D], BF16, tag="vbs")
                warm_w = const.tile([128, 128], BF16, tag="warm_w")

                # Per-engine program-order chains. The Tile scheduler orders
                # by its own dependency heuristics; these order-only edges pin
                # each engine's queue to the pipeline order designed here.
                _last = {}

                def chain(key, instr):
                    prev = _last.get(key)
                    if prev is not None:
                        tile.add_dep_helper(
                            instr.ins, prev.ins, sync=False, reason="order"
                        )
                    _last[key] = instr
                    return instr

                chain("dve", nc.vector.memset(warm_w[:], 0.0))
                chain("dve", nc.vector.memset(ones_s[:], 1.0))

                # PE warmup: the cost model's p-state ramp clock starts at
                # the first matmul and never resets, so a single early matmul
                # (during the DMA lead-in) makes every loop matmul full-rate.
                warm_ps = sps.tile([128, HW], F32, tag="sp", name="warm_ps")
                for _w in range(warm):
                    chain("pe", nc.tensor.matmul(
                        warm_ps[:, 0:128], lhsT=warm_w[:], rhs=warm_w[:],
                        start=True, stop=True,
                    ))

                # Input DMAs. HWDGE (sync) queue carries the q chunks in need
                # order; the SWDGE (gpsimd) queue carries kT tile 0 + bias +
                # V in parallel so the first QK unblocks as early as possible.
                chain("pool", nc.gpsimd.dma_start(bias_s[:], biasb[:]))
                chain("sp", nc.sync.dma_start(qTs[:, 0:512], qT[:, 0:512]))
                chain("sp", nc.sync.dma_start(kTs[:, 0:128], kT[:, 0:128]))
                chain("sp", nc.sync.dma_start(qTs[:, 512:1024],
                                              qT[:, 512:1024]))
                chain("pool", nc.gpsimd.dma_start(sbias_s[:], sbias[:]))
                chain("sp", nc.sync.dma_start(kTs[:, 128:512], kT[:, 128:512]))
                chain("pool", nc.gpsimd.dma_start(vbs[:, 0:4, :], vb[:, 0:512]))
                chain("sp", nc.sync.dma_start(kTs[:, 512:2048],
                                              kT[:, 512:2048]))
                chain("pool", nc.gpsimd.dma_start(vbs[:, 4:16, :],
                                                  vb[:, 512:2048]))
                chain("sp", nc.sync.dma_start(qTs[:, 1024:2048],
                                              qT[:, 1024:2048]))

                # four independent 1-bank PSUM accumulators: o_q[h][c] holds
                # the PV accumulation for half h, q-chunk c; Z for half h is
                # reduced into the opposite half's pair (idle while h runs).
                # Separate tiles keep the dependency streams disjoint.
                o_q = [
                    [
                        ops.tile([128, 512], F32, tag=f"o{h}{c}",
                                 name=f"o{h}{c}")
                        for c in range(2)
                    ]
                    for h in range(2)
                ]

                def mm_qk(h, ki):
                    sp = sps.tile([128, HW], F32, tag="sp", name=f"sp{h}_{ki}")
                    for c in range(2):
                        chain("pe", nc.tensor.matmul(
                            sp[:, c * 512 : (c + 1) * 512],
                            lhsT=kTs[:, ki * 128 : (ki + 1) * 128],
                            rhs=qTs[:, h * HW + c * 512 : h * HW + (c + 1) * 512],
                            start=True,
                            stop=True,
                        ))
                    return sp

                from collections import deque
                pv_q = deque()  # deferred PV chunk emissions (thunks)

                def flush_pv(budget):
                    while pv_q and budget > 0:
                        pv_q.popleft()()
                        budget -= 1

                def mm_pv(h, ki, p, c0=0, c1=2, defer=False):
                    for c in range(c0, c1):
                        def emit(h=h, ki=ki, p=p, c=c):
                            chain("pe", nc.tensor.matmul(
                                o_q[h][c][:],
                                lhsT=vbs[:, ki, :],
                                rhs=p[:, c * 512 : (c + 1) * 512],
                                start=(ki == 0),
                                stop=(ki == KT - 1),
                            ))
                        if defer:
                            pv_q.append(emit)
                        else:
                            emit()

                def mm_z(dst, src, start, stop, c0=0, c1=2):
                    # partition-reduce src into dst (list of two [128, 512]
                    # psum APs, or a [128, 1024] tile sliced per chunk)
                    for c in range(c0, c1):
                        d = (dst[c] if isinstance(dst, list)
                             else dst[:, c * 512 : (c + 1) * 512])
                        chain("pe", nc.tensor.matmul(
                            d,
                            lhsT=ones_s[:],
                            rhs=src[:, c * 512 : (c + 1) * 512],
                            start=start,
                            stop=stop,
                        ))

                def exp_act(h, ki, sp, p, c0=0, c1=2):
                    # one ACT instr over chunks [c0, c1) (shared bias column)
                    chain("act", nc.scalar.activation(
                        p[:, c0 * 512 : c1 * 512],
                        sp[:, c0 * 512 : c1 * 512],
                        EXP,
                        bias=bias_s[:, ki : ki + 1],
                        scale=1.0,
                    ))

                def exp_dve(h, ki, sp, p):
                    # Schraudolph bf16 exp on the DVE (one tensor_scalar)
                    chain("dve", nc.vector.tensor_scalar(
                        p[:].bitcast(U16),
                        sp[:],
                        SCH_A,
                        sbias_s[:, ki : ki + 1],
                        MULT,
                        ADD,
                    ))

                def tail(h, rz, oTs, zsrc, c, queue=None):
                    # reciprocal -> normalize -> store for 512-wide q chunk c
                    # of half h. zsrc = where this half's Z was reduced.
                    sl = slice(c * 512, (c + 1) * 512)
                    z = zsrc[c] if isinstance(zsrc, list) else zsrc[:, sl]
                    chain("dve", nc.vector.reciprocal(rz[:, sl], z))
                    chain("dve", nc.vector.tensor_mul(
                        oTs[:, sl], o_q[h][c][:], rz[:, sl]
                    ))
                    qkey = "pool" if queue is nc.gpsimd else "sp"
                    chain(qkey, (queue or nc.sync).dma_start(
                        oT[:, h * HW + c * 512 : h * HW + (c + 1) * 512],
                        oTs[:, sl],
                    ))

                # Half A folds all 16 p-tiles (its tail overlaps half B's
                # loop, so the fold latency is free) and Z-reduces with one
                # ones-matmul per chunk into half B's accumulators BEFORE
                # B's PV chain starts. So that A's Z -> recip never stalls
                # B's pipeline, B's first three PV pairs are deferred past
                # the reciprocal and drained at ~1 extra chunk per tile;
                # QK feeds always lead each block so ACT never starves.
                # Half B folds only 0..nfold-1; its last fz p-tiles join Z
                # via PSUM-accumulated ones-matmuls into half A's (long
                # dead) accumulators, so nothing remains after the final
                # exp but one short Z->recip->mul->DMA pipe per chunk.
                nfold = KT - fz
                sp_tiles = {}
                sp_tiles[(0, 0)] = mm_qk(0, 0)
                sp_tiles[(0, 1)] = mm_qk(0, 1)
                pend_a = [None]
                for h in range(2):
                    last = h == 1
                    nf = KT if h == 0 else nfold
                    p_tiles = {}
                    acc = None
                    rz = tails.tile([128, HW], F32, tag="rz", name=f"rz{h}")
                    oTs = tails.tile([128, HW], F32, tag="oTs", name=f"oTs{h}")
                    for ki in range(KT):
                        closing = last and ki == KT - 1
                        sp = sp_tiles.pop((h, ki))
                        p = pp.tile([128, HW], BF16, tag="p", name=f"p{h}_{ki}")
                        p_tiles[ki] = p
                        # --- exp (the closing tile is chunked so the final
                        # Z->recip->mul->DMA pipeline starts half a tile early)
                        if not closing:
                            if (h, ki) in dve_off:
                                exp_dve(h, ki, sp, p)
                            else:
                                exp_act(h, ki, sp, p)
                        else:
                            # sub-order [0:512], [896:1024], [512:896]: all
                            # writes to each o-bank land before any tail read,
                            # so per-tile WAR tracking can't serialize them
                            subs = []
                            for a, b, tg in ((0, 512, "clA"), (896, 1024, "clC"),
                                             (512, 896, "clB")):
                                ps = pp.tile([128, b - a], BF16, tag=tg,
                                             name=f"pcl{a}")
                                chain("act", nc.scalar.activation(
                                    ps[:, 0 : b - a],
                                    sp[:, a:b],
                                    EXP,
                                    bias=bias_s[:, ki : ki + 1],
                                    scale=1.0,
                                ))
                                subs.append((a, b, ps))
                        # --- PE: QK feed first (ACT must never starve), then
                        # PV work (deferred around the boundary), then Z work
                        if ki + 2 < KT:
                            sp_tiles[(h, ki + 2)] = mm_qk(h, ki + 2)
                        elif not last:
                            sp_tiles[(1, ki + 2 - KT)] = mm_qk(1, ki + 2 - KT)
                        if closing:
                            zb = o_q[0]  # A's accumulators: long since read
                            flush_pv(99)
                            if nf == KT - 1:
                                mm_z(zb, acc, True, False)
                            # Z + PV matmuls per sub (writes into the o-banks
                            # first), then the tail reads in bank order
                            for (a, b, ps) in subs:
                                c = a // 512
                                osl = slice(a - c * 512, b - c * 512)
                                w = b - a
                                chain("pe", nc.tensor.matmul(
                                    zb[c][:, osl], lhsT=ones_s[:],
                                    rhs=ps[:, 0:w], start=False, stop=True,
                                ))
                                chain("pe", nc.tensor.matmul(
                                    o_q[h][c][:, osl], lhsT=vbs[:, ki, :],
                                    rhs=ps[:, 0:w], start=False, stop=True,
                                ))
                                if a == 0:
                                    # chunk 0 normalizes as soon as its bank
                                    # closes; its store rides the SWDGE queue
                                    chain("dve", nc.vector.reciprocal(
                                        rz[:, 0:512], zb[0][:]))
                                    chain("dve", nc.vector.tensor_mul(
                                        oTs[:, 0:512], o_q[h][0][:],
                                        rz[:, 0:512],
                                    ))
                                    chain("pool", nc.gpsimd.dma_start(
                                        oT[:, h * HW : h * HW + 512],
                                        oTs[:, 0:512],
                                    ))
                            # bank 1 tail: both sub-ranges' writes are done
                            for a, b in ((512, 896), (896, 1024)):
                                sl = slice(a, b)
                                osl = slice(a - 512, b - 512)
                                chain("dve", nc.vector.reciprocal(
                                    rz[:, sl], zb[1][:, osl]))
                            for a, b in ((512, 896), (896, 1024)):
                                sl = slice(a, b)
                                osl = slice(a - 512, b - 512)
                                chain("dve", nc.vector.tensor_mul(
                                    oTs[:, sl], o_q[h][1][:, osl], rz[:, sl]
                                ))
                                chain("sp", nc.sync.dma_start(
                                    oT[:, h * HW + a : h * HW + b],
                                    oTs[:, sl],
                                ))
                            continue
                        if last and ki == 0 and pend_a[0] is not None:
                            pend_a[0]()  # half A's Z + reciprocal
                        mm_pv(h, ki, p, defer=last)
                        # B tiles 0-1: emit no PVs (they must trail A's
                        # reciprocal read of B's accumulators); then drain
                        if not (last and ki <= 1):
                            flush_pv(3)
                        if last and ki == nf:
                            mm_z(o_q[0], acc, True, False)
                        if last and nf <= ki < KT - 1:
                            mm_z(o_q[0], p, False, False)
                        if last and ki == 2 and pend_a[0] is not None:
                            pend_a[1]()  # half A's normalize + store
                        # --- DVE fold chain (bf16 2x mode)
                        if ki == 1:
                            acc = accp.tile([128, HW], BF16, tag="acc",
                                            name=f"acc{h}_{ki}")
                            chain("dve", nc.vector.tensor_add(
                                acc[:], p_tiles[0][:], p_tiles[1][:]
                            ))
                        elif 2 <= ki <= nf - 1:
                            nacc = accp.tile([128, HW], BF16, tag="acc",
                                             name=f"acc{h}_{ki}")
                            chain("dve", nc.vector.tensor_add(
                                nacc[:], acc[:], p[:]
                            ))
                            acc = nacc
                        if h == 0 and ki == KT - 1:
                            acc_a, rz_a, oTs_a = acc, rz, oTs

                            def _a_z():
                                # Z for half A: one ones-matmul per chunk
                                # into B's accumulators, then reciprocal;
                                # B's PVs re-init those banks afterwards.
                                mm_z(o_q[1], acc_a, True, True)
                                sl0, sl1 = slice(0, 512), slice(512, 1024)
                                chain("dve", nc.vector.reciprocal(
                                    rz_a[:, sl0], o_q[1][0][:]))
                                chain("dve", nc.vector.reciprocal(
                                    rz_a[:, sl1], o_q[1][1][:]))

                            def _a_norm():
                                for c in range(2):
                                    sl = slice(c * 512, (c + 1) * 512)
                                    chain("dve", nc.vector.tensor_mul(
                                        oTs_a[:, sl], o_q[0][c][:], rz_a[:, sl]
                                    ))
                                    chain("sp", nc.sync.dma_start(
                                        oT[:, c * 512 : (c + 1) * 512],
                                        oTs_a[:, sl],
                                    ))

                            pend_a[0] = _a_z
                            pend_a.append(_a_norm)

            for _rep in range(reps):
                _emit_body()

    _split_multi_waits(nc)
    return nc


def _bias_kernel(waveforms, gains, window, atom_indices, shifts) -> np.ndarray:
    waveforms = np.asarray(waveforms, dtype=np.float32)
    gains = np.asarray(gains, dtype=np.float32)
    window = np.asarray(window, dtype=np.float32)
    atom_indices = np.asarray(atom_indices).astype(np.int64)
    shifts = np.asarray(shifts).astype(np.int64)
    atoms = waveforms[atom_indices, :S]                  # [P, S]
    bases = atoms * gains[:, None]                       # [P, S]
    shifted = np.stack(
        [np.roll(bases[p], shifts[p]) for p in range(bases.shape[0])]
    )
    return (shifted * window[None, :S]).sum(0).astype(np.float32)  # [S]


def _host_inputs(queries, keys, values, waveforms, gains, window,
                 atom_indices, shifts):
    """Per-batch DRAM images + replicated small tensors."""
    import ml_dtypes

    queries = np.asarray(queries, dtype=np.float32)
    keys = np.asarray(keys, dtype=np.float32)
    values = np.asarray(values, dtype=np.float32)

    bias = _bias_kernel(waveforms, gains, window, atom_indices, shifts)
    shifted = bias - C_SHIFT
    biasb = np.ascontiguousarray(shifted.reshape(KT, 128).T)      # [128, KT]
    sbias = np.ascontiguousarray(
        (SCH_A * shifted + SCH_B0).reshape(KT, 128).T
    ).astype(np.float32)                                          # [128, KT]

    in_maps = []
    for b in range(B):
        vbf = values[b].astype(ml_dtypes.bfloat16)                # [S, D]
        vb = np.ascontiguousarray(
            vbf.reshape(KT, 128, D).transpose(1, 0, 2).reshape(128, KT * D)
        )
        in_maps.append(
            {
                "qT": np.ascontiguousarray(queries[b].T),
                "kT": np.ascontiguousarray(keys[b].T),
                "vb": vb,
                "biasb": biasb,
                "sbias": sbias,
            }
        )
    return in_maps


def kernel(queries, keys, values, waveforms, gains, window, atom_indices,
           shifts):
    in_maps = _host_inputs(
        queries, keys, values, waveforms, gains, window, atom_indices, shifts
    )
    nc = build_nc()
    res = run_bass_kernel_spmd(nc, in_maps, list(range(B)))
    out = np.stack(
        [np.ascontiguousarray(res.results[b]["oT"].T) for b in range(B)]
    )
    return out.astype(np.float32)


# revision 46
# speedup vs baseline: 1.2226x; 1.0495x over previous
"""Trainium2 Bass kernel for nn_AttentionBasisSynthesizer.

out[b] = softmax(Q[b] @ K[b].T + bias) @ V[b], bias[k] built from a tiny
sinusoidal atom bank (computed on host, replicated to every core).

Sharding: data-parallel over the batch dim — 8 batches onto 8 NeuronCores,
one batch per core. Each core computes its full [S, S] attention.

Device-side layout (per core): scores are computed TRANSPOSED, sT[k, q], so
- the key-dim bias is per-partition and folds into the ACT engine's free
  exp(scale*x + bias) affine,
- P @ V needs no transposes: out.T[d, q] = sum over k-tiles of
  matmul(lhsT=V_tile[k,d], rhs=exp_scores[k, q]).
Softmax uses a constant shift C instead of a per-row max (mathematically
exact; scores ~ N(0, sqrt(128)) so exp(s - C) can neither overflow bf16 nor
flush the row maximum for any plausible data).

Structure (engine balance + head/tail overlap):
- The q axis is processed in two halves of 1024. PSUM: four [128, 512]
  PV accumulator banks o_q[h][c] + two [128, 1024] score tiles in
  rotation = exactly 8 banks.
- Per (half, k-tile): QK (2x N=512 fp32r matmuls) -> exp -> PV (2x N=512
  bf16 matmuls). Emission pins each engine's queue order (the Tile
  scheduler is otherwise free-order): QK feeds lead every block so the
  ACT engine — the global bottleneck — never starves.
- exp work is balanced across ACT and DVE: on SPLIT tiles, ACT computes
  q-chunk 0 while the DVE computes q-chunk 1 concurrently via the
  Schraudolph bit-trick, bf16(e^x) ~= bitcast_bf16(uint16(A*x + B_k)),
  one tensor_scalar (mult-imm, add-per-partition-vec) into a separate
  tile (the f32->uint16 saturating convert clamps underflow to +0).
  DVE exps are emitted one block early so they overlap the previous
  tile's ACT exp.
- Z[q] = sum_k p[k,q] runs as two per-chunk bf16 fold chains on the DVE;
  the final 128-partition reduction is a ones-matmul into idle PSUM.
  Half A's lands in half B's accumulators just before B's PVs start
  (B's first PV pairs are deferred past A's reciprocal read); half B's
  lands in half A's long-dead accumulators.
- Half A's tail (reciprocal, normalize, store) overlaps half B's main
  loop; half B's closing tile is cut into 512/384/128 sub-chunks so the
  kernel's last dependence chain (Z -> recip -> mul -> DMA) is short.
- V is converted to bf16 and laid out [k%128, k//128, d] on the host.
"""

import numpy as np

import concourse.bass as bass
import concourse.tile as tile
from concourse import mybir
from concourse.bass_utils import run_bass_kernel_spmd
from concourse.vector_clock import ScopedClock

B, S, D = 8, 2048, 128
KT = S // 128           # 16 key tiles of 128
HW = S // 2             # half width (q) = 1024
C_SHIFT = 20.0          # constant softmax shift (exact: softmax(x-C)=softmax(x))

# Schraudolph exp constants for bf16 (unit in last place of exponent = 128):
# e^x ~= bitcast_bf16(uint16(A*x + B)); B = 127*128 - C0, C0 tuned minimax.
SCH_A = 128.0 / float(np.log(2.0))
SCH_B0 = 128.0 * 127.0 - 5.5

F32 = mybir.dt.float32
F32R = mybir.dt.float32r
F16 = mybir.dt.float16
BF16 = mybir.dt.bfloat16
U16 = mybir.dt.uint16
EXP = mybir.ActivationFunctionType.Exp
MULT = mybir.AluOpType.mult
ADD = mybir.AluOpType.add

# per-half k-tiles whose exp runs entirely on the DVE via the bit-trick,
# with their QK scores staged in the opposite half's idle accumulator
# banks (so they leave the score-tile rotation, and the ACT engine skips
# them without a bubble). Half A tiles must clear before the boundary Z;
# half B tiles must start after half A's normalize frees the banks.
OFF_A = (4, 9)
OFF_B = (5, 9)
WARM = 1                # PE warmup matmul count
FZ = 2                  # trailing p-tiles per half reduced via Z-matmul
                        # instead of the DVE fold chain


def _install_tile_drain_patch():
    """This container's walrus accepts only one semaphore wait per sync-queue
    instruction, but TileContext's tail drain carries one wait per tracked
    proc. Split the waits across single-wait NOPs ahead of the drain (the
    sync queue is in-order, so the drain still begins only after every wait
    has been satisfied)."""

    def _drain_and_barrier(self, tick_clock, wait_clock):
        nc = self.nc
        probe = nc.sync.nop()
        wait_clock.add_sem_waits(
            probe.ins, ScopedClock({None: tick_clock.global_clock})
        )
        si = probe.ins.sync_info
        waits = list(si.on_wait or []) if si is not None else []
        if len(waits) > 1:
            si.on_wait = waits[:1]
            # distribute the remaining waits across all engine queues so the
            # checks evaluate in parallel; the following all-engine barrier
            # joins them back together.
            engines = [nc.sync, nc.scalar, nc.vector, nc.gpsimd, nc.tensor]
            for i, w in enumerate(waits[1:]):
                extra = engines[i % len(engines)].nop()
                extra.ins.sync_info = mybir.SyncInfo(on_wait=[w], on_update=[])
        nc.sync.drain()
        nc.all_engine_barrier()
        assert self.sems is not None
        popped = nc._tile_sem_poison_stack.pop()
        assert popped is self._sem_poison
        nc.clear_and_free_semaphores(list(self.sems.allocated().values()))
        nc.all_engine_barrier()

    tile.TileContext._drain_and_barrier = _drain_and_barrier


def _split_multi_waits(nc: bass.Bass, limit: int = 1) -> int:
    """This container's walrus rejects instructions carrying more than one
    semaphore wait ("Too many sync wait commands"). Hoist excess waits onto
    same-engine NOPs inserted immediately before the instruction — engine
    queues dispatch in order, so the semantics are identical."""
    n_split = 0
    for fn in nc.m.functions:
        for blk in fn.blocks:
            insts = blk.instructions
            out = []
            for inst in insts:
                si = inst.sync_info
                waits = list(si.on_wait or []) if si is not None else []
                if len(waits) > limit:
                    keep = waits[:limit]
                    extra = waits[limit:]
                    for j in range(0, len(extra), limit):
                        nop = mybir.InstNoOp(
                            name=f"{inst.name}-waitsplit{j}",
                            ins=[],
                            outs=[],
                            engine=inst.engine,
                        )
                        nop.sync_info = mybir.SyncInfo(
                            on_wait=extra[j : j + limit], on_update=[]
                        )
                        nc.register_instruction(nop, overwrite=True)
                        out.append(nop)
                        n_split += 1
                    si.on_wait = keep
                out.append(inst)
            if n_split:
                blk.instructions = out
    return n_split


def build_nc(reps: int = 1, warm: int = WARM, off_a=OFF_A, off_b=OFF_B,
             fz: int = FZ) -> bass.Bass:
    """reps>1 unrolls the whole body (including input loads) that many times
    inside one NEFF — used only by the test harness to measure steady-state
    per-execution HW time as a marginal; the graded path uses reps=1."""
    _install_tile_drain_patch()
    offs = (frozenset(off_a), frozenset(off_b))
    nc = bass.Bass()

    qT = nc.declare_dram_parameter("qT", [D, S], F16, isOutput=False)
    kT = nc.declare_dram_parameter("kT", [D, S], F16, isOutput=False)
    vb = nc.declare_dram_parameter("vb", [128, KT * D], BF16, isOutput=False)
    biasb = nc.declare_dram_parameter("biasb", [128, KT], F32, isOutput=False)
    sbias = nc.declare_dram_parameter("sbias", [128, KT], F32, isOutput=False)
    oT = nc.declare_dram_parameter("oT", [D, S], F32, isOutput=True)

    with tile.TileContext(nc) as tc:
        with (
            tc.tile_pool(name="const", bufs=1) as const,
            tc.tile_pool(name="pp", bufs=6) as pp,
            tc.tile_pool(name="accp", bufs=3) as accp,
            tc.tile_pool(name="tails", bufs=2) as tails,
            tc.tile_pool(name="sps", bufs=2, space="PSUM") as sps,
            tc.tile_pool(name="ops", bufs=1, space="PSUM") as ops,
        ):
            def _emit_body():
                kTs = const.tile([D, S], F16, tag="kTs")
                qTs = const.tile([D, S], F16, tag="qTs")
                bias_s = const.tile([128, KT], F32, tag="bias")
                sbias_s = const.tile([128, KT], F32, tag="sbias")
                ones_s = const.tile([128, 128], BF16, tag="ones")
                vbs = const.tile([128, KT, D], BF16, tag="vbs")
                warm_w = const.tile([128, 128], BF16, tag="warm_w")

                # Per-engine program-order chains. The Tile scheduler orders
                # by its own dependency heuristics; these order-only edges pin
                # each engine's queue to the pipeline order designed here.
                _last = {}

                def chain(key, instr):
                    prev = _last.get(key)
                    if prev is not None:
                        tile.add_dep_helper(
                            instr.ins, prev.ins, sync=False, reason="order"
                        )
                    _last[key] = instr
                    return instr

                chain("dve", nc.vector.memset(warm_w[:], 0.0))
                chain("dve", nc.vector.memset(ones_s[:], 1.0))

                # PE warmup: the cost model's p-state ramp clock starts at
                # the first matmul and never resets, so a single early matmul
                # (during the DMA lead-in) makes every loop matmul full-rate.
                warm_ps = sps.tile([128, HW], F32, tag="sp", name="warm_ps")
                for _w in range(warm):
                    chain("pe", nc.tensor.matmul(
                        warm_ps[:, 0:128], lhsT=warm_w[:], rhs=warm_w[:],
                        start=True, stop=True,
                    ))

                # Input DMAs. HWDGE (sync) queue carries the QK operands in
                # need order; the SWDGE (gpsimd) queue carries bias + V in
                # parallel so the first exp unblocks as early as possible.
                chain("pool", nc.gpsimd.dma_start(bias_s[:], biasb[:]))
                chain("sp", nc.sync.dma_start(qTs[:, 0:512], qT[:, 0:512]))
                chain("sp", nc.sync.dma_start(kTs[:, 0:128], kT[:, 0:128]))
                chain("sp", nc.sync.dma_start(qTs[:, 512:1024],
                                              qT[:, 512:1024]))
                chain("pool", nc.gpsimd.dma_start(sbias_s[:], sbias[:]))
                chain("sp", nc.sync.dma_start(kTs[:, 128:512], kT[:, 128:512]))
                chain("pool", nc.gpsimd.dma_start(vbs[:, 0:4, :], vb[:, 0:512]))
                chain("sp", nc.sync.dma_start(kTs[:, 512:2048],
                                              kT[:, 512:2048]))
                chain("pool", nc.gpsimd.dma_start(vbs[:, 4:16, :],
                                                  vb[:, 512:2048]))
                chain("sp", nc.sync.dma_start(qTs[:, 1024:2048],
                                              qT[:, 1024:2048]))

                # four independent 1-bank PSUM accumulators: o_q[h][c] holds
                # the PV accumulation for half h, q-chunk c. Separate tiles
                # keep the dependency streams disjoint (tracking is per-tile).
                o_q = [
                    [
                        ops.tile([128, 512], F32, tag=f"o{h}{c}",
                                 name=f"o{h}{c}")
                        for c in range(2)
                    ]
                    for h in range(2)
                ]

                def mm_qk(h, ki):
                    sp = sps.tile([128, HW], F32, tag="sp", name=f"sp{h}_{ki}")
                    for c in range(2):
                        chain("pe", nc.tensor.matmul(
                            sp[:, c * 512 : (c + 1) * 512],
                            lhsT=kTs[:, ki * 128 : (ki + 1) * 128],
                            rhs=qTs[:, h * HW + c * 512 : h * HW + (c + 1) * 512],
                            start=True,
                            stop=True,
                        ))
                    return sp

                p_map = {}

                def make_p(h, ki):
                    if (h, ki) not in p_map:
                        p_map[(h, ki)] = pp.tile([128, HW], BF16, tag="p",
                                                 name=f"p{h}_{ki}")
                    return p_map[(h, ki)]

                def csrc(h, ki, c):
                    # AP of tile (h, ki)'s q-chunk c exp values
                    return make_p(h, ki)[:, c * 512 : (c + 1) * 512]

                sp_tiles = {}

                def emit_off(h, ki):
                    # DVE-offloaded tile: QK scores go into the opposite
                    # half's idle accumulator banks (not the sp rotation),
                    # then one Schraudolph tensor_scalar produces the bf16
                    # exp tile. The ACT engine never sees this tile.
                    scr = o_q[1 - h]
                    for c in range(2):
                        chain("pe", nc.tensor.matmul(
                            scr[c][:],
                            lhsT=kTs[:, ki * 128 : (ki + 1) * 128],
                            rhs=qTs[:, h * HW + c * 512 : h * HW + (c + 1) * 512],
                            start=True,
                            stop=True,
                        ))
                    p = make_p(h, ki)
                    for c in range(2):
                        chain("dve", nc.vector.tensor_scalar(
                            p[:, c * 512 : (c + 1) * 512].bitcast(U16),
                            scr[c][:],
                            SCH_A,
                            sbias_s[:, ki : ki + 1],
                            MULT,
                            ADD,
                        ))

                from collections import deque
                pv_q = deque()  # deferred PV chunk emissions (thunks)

                def flush_pv(budget):
                    while pv_q and budget > 0:
                        pv_q.popleft()()
                        budget -= 1

                def mm_pv(h, ki, c0=0, c1=2, defer=False):
                    for c in range(c0, c1):
                        def emit(h=h, ki=ki, c=c):
                            chain("pe", nc.tensor.matmul(
                                o_q[h][c][:],
                                lhsT=vbs[:, ki, :],
                                rhs=csrc(h, ki, c),
                                start=(ki == 0),
                                stop=(ki == KT - 1),
                            ))
                        if defer:
                            pv_q.append(emit)
                        else:
                            emit()

                def mm_z(dst, srcs, start, stop, c0=0, c1=2):
                    # partition-reduce per-chunk sources into dst (a list of
                    # two [128, 512] psum tiles)
                    for c in range(c0, c1):
                        chain("pe", nc.tensor.matmul(
                            dst[c][:],
                            lhsT=ones_s[:],
                            rhs=srcs[c],
                            start=start,
                            stop=stop,
                        ))

                def exp_act(h, ki, sp):
                    # ACT exp: full tile, or chunk 0 only for SPLIT tiles
                    make_p(h, ki)
                    v = p_map[(h, ki)]
                    if isinstance(v, tuple):
                        out, src = v[0][:, 0:512], sp[:, 0:512]
                    else:
                        out, src = v[:, 0:HW], sp[:, 0:HW]
                    chain("act", nc.scalar.activation(
                        out, src, EXP, bias=bias_s[:, ki : ki + 1], scale=1.0,
                    ))

                # Half A folds all 16 p-tiles (its tail overlaps half B's
                # loop, so the fold latency is free) and Z-reduces with one
                # ones-matmul per chunk into half B's accumulators BEFORE
                # B's PV chain starts; B's first PV pairs are deferred past
                # A's reciprocal read and drained a chunk at a time. Half B
                # folds only 0..nfold-1; its last fz p-tiles join Z via
                # PSUM-accumulated ones-matmuls into half A's (long dead)
                # accumulators, so nothing remains after the final exp but
                # one short Z->recip->mul->DMA pipe per closing sub-chunk.
                nfold = KT - fz
                # per-half sequences of ACT-handled tiles; offloaded tiles
                # leave the sp rotation entirely
                seqs = [
                    [ki for ki in range(KT) if ki not in offs[h]]
                    for h in range(2)
                ]
                sp_tiles[(0, seqs[0][0])] = mm_qk(0, seqs[0][0])
                sp_tiles[(0, seqs[0][1])] = mm_qk(0, seqs[0][1])
                pend_a = [None]
                for h in range(2):
                    last = h == 1
                    seq = seqs[h]
                    nf = KT if h == 0 else nfold
                    acc = [None, None]
                    folded = 0  # fold chain caught up through ki < folded
                    rz = tails.tile([128, HW], F32, tag="rz", name=f"rz{h}")
                    oTs = tails.tile([128, HW], F32, tag="oTs", name=f"oTs{h}")
                    for j, ki in enumerate(seq):
                        closing = last and ki == KT - 1
                        sp = sp_tiles[(h, ki)]
                        # --- exp (the closing tile is chunked so the final
                        # Z->recip->mul->DMA pipeline starts early)
                        if not closing:
                            exp_act(h, ki, sp)
                        else:
                            # sub-order [0:512], [896:1024], [512:896]: all
                            # writes to each o-bank land before any tail
                            # read, so per-tile WAR tracking can't serialize
                            subs = []
                            for a, b, tg in ((0, 512, "clA"),
                                             (896, 1024, "clC"),
                                             (512, 896, "clB")):
                                ps = pp.tile([128, b - a], BF16, tag=tg,
                                             name=f"pcl{a}")
                                chain("act", nc.scalar.activation(
                                    ps[:, 0 : b - a],
                                    sp[:, a:b],
                                    EXP,
                                    bias=bias_s[:, ki : ki + 1],
                                    scale=1.0,
                                ))
                                subs.append((a, b, ps))
                        # --- PE: QK feed first (ACT must never starve), then
                        # offloaded-tile work, then PV work, then Z work
                        if j + 2 < len(seq):
                            nk = seq[j + 2]
                            sp_tiles[(h, nk)] = mm_qk(h, nk)
                        elif not last:
                            nk = seqs[1][j + 2 - len(seq)]
                            sp_tiles[(1, nk)] = mm_qk(1, nk)
                        if ki + 1 in offs[h]:
                            emit_off(h, ki + 1)
                        if closing:
                            zb = o_q[0]  # A's accumulators: long since read
                            flush_pv(99)
                            if nf == KT - 1:
                                mm_z(zb, (acc[0][:], acc[1][:]), True, False)
                            # Z matmuls per sub first (they gate the recips),
                            # then PVs, then the tail reads in bank order
                            for (a, b, ps) in subs:
                                c = a // 512
                                osl = slice(a - c * 512, b - c * 512)
                                w = b - a
                                chain("pe", nc.tensor.matmul(
                                    zb[c][:, osl], lhsT=ones_s[:],
                                    rhs=ps[:, 0:w], start=False, stop=True,
                                ))
                                if a == 0:
                                    chain("dve", nc.vector.reciprocal(
                                        rz[:, 0:512], zb[0][:]))
                                chain("pe", nc.tensor.matmul(
                                    o_q[h][c][:, osl], lhsT=vbs[:, ki, :],
                                    rhs=ps[:, 0:w], start=False, stop=True,
                                ))
                                if a == 0:
                                    # chunk 0 normalizes as soon as its bank
                                    # closes; its store rides the SWDGE queue
                                    chain("dve", nc.vector.tensor_mul(
                                        oTs[:, 0:512], o_q[h][0][:],
                                        rz[:, 0:512],
                                    ))
                                    chain("pool", nc.gpsimd.dma_start(
                                        oT[:, h * HW : h * HW + 512],
                                        oTs[:, 0:512],
                                    ))
                            # bank 1 tail: both sub-ranges' writes are done
                            for a, b in ((512, 896), (896, 1024)):
                                sl = slice(a, b)
                                osl = slice(a - 512, b - 512)
                                chain("dve", nc.vector.reciprocal(
                                    rz[:, sl], zb[1][:, osl]))
                            for a, b in ((512, 896), (896, 1024)):
                                sl = slice(a, b)
                                osl = slice(a - 512, b - 512)
                                chain("dve", nc.vector.tensor_mul(
                                    oTs[:, sl], o_q[h][1][:, osl], rz[:, sl]
                                ))
                                chain("sp", nc.sync.dma_start(
                                    oT[:, h * HW + a : h * HW + b],
                                    oTs[:, sl],
                                ))
                            continue
                        if last and ki == 0 and pend_a[0] is not None:
                            pend_a[0]()  # half A's Z + reciprocal
                        mm_pv(h, ki, defer=last)
                        if ki - 1 in offs[h]:
                            mm_pv(h, ki - 1, defer=last)
                        # B tiles 0-1: emit no PVs (they must trail A's
                        # reciprocal read of B's accumulators); then drain
                        if not (last and ki <= 1):
                            flush_pv(3)
                        if last and ki == nf:
                            mm_z(o_q[0], (acc[0][:], acc[1][:]), True, False)
                        if last and nf <= ki < KT - 1:
                            mm_z(o_q[0], (csrc(h, ki, 0), csrc(h, ki, 1)),
                                 False, False)
                        if last and ki == 2 and pend_a[0] is not None:
                            pend_a[1]()  # half A's normalize + store
                        # --- DVE fold chains (bf16 2x mode), one per
                        # q-chunk, catching up over offloaded tiles
                        while folded <= min(ki, nf - 1):
                            kk = folded
                            if kk == 1:
                                for c in range(2):
                                    acc[c] = accp.tile([128, 512], BF16,
                                                       tag=f"acc{c}",
                                                       name=f"acc{c}_{h}_{kk}")
                                    chain("dve", nc.vector.tensor_add(
                                        acc[c][:], csrc(h, 0, c), csrc(h, 1, c)
                                    ))
                            elif kk >= 2:
                                for c in range(2):
                                    nacc = accp.tile([128, 512], BF16,
                                                     tag=f"acc{c}",
                                                     name=f"acc{c}_{h}_{kk}")
                                    chain("dve", nc.vector.tensor_add(
                                        nacc[:], acc[c][:], csrc(h, kk, c)
                                    ))
                                    acc[c] = nacc
                            folded += 1
                        if h == 0 and ki == KT - 1:
                            acc_a = (acc[0], acc[1])
                            rz_a, oTs_a = rz, oTs

                            def _a_z():
                                # Z for half A: one ones-matmul per chunk
                                # into B's accumulators, then reciprocal;
                                # B's PVs re-init those banks afterwards.
                                mm_z(o_q[1], (acc_a[0][:], acc_a[1][:]),
                                     True, True)
                                sl0, sl1 = slice(0, 512), slice(512, 1024)
                                chain("dve", nc.vector.reciprocal(
                                    rz_a[:, sl0], o_q[1][0][:]))
                                chain("dve", nc.vector.reciprocal(
                                    rz_a[:, sl1], o_q[1][1][:]))

                            def _a_norm():
                                for c in range(2):
                                    sl = slice(c * 512, (c + 1) * 512)
                                    chain("dve", nc.vector.tensor_mul(
                                        oTs_a[:, sl], o_q[0][c][:], rz_a[:, sl]
                                    ))
                                    chain("sp", nc.sync.dma_start(
                                        oT[:, c * 512 : (c + 1) * 512],
                                        oTs_a[:, sl],
                                    ))

                            pend_a[0] = _a_z
                            pend_a.append(_a_norm)

            for _rep in range(reps):
                _emit_body()

    _split_multi_waits(nc)
    return nc


def _bias_kernel(waveforms, gains, window, atom_indices, shifts) -> np.ndarray:
    waveforms = np.asarray(waveforms, dtype=np.float32)
    gains = np.asarray(gains, dtype=np.float32)
    window = np.asarray(window, dtype=np.float32)
    atom_indices = np.asarray(atom_indices).astype(np.int64)
    shifts = np.asarray(shifts).astype(np.int64)
    atoms = waveforms[atom_indices, :S]                  # [P, S]
    bases = atoms * gains[:, None]                       # [P, S]
    shifted = np.stack(
        [np.roll(bases[p], shifts[p]) for p in range(bases.shape[0])]
    )
    return (shifted * window[None, :S]).sum(0).astype(np.float32)  # [S]


def _host_inputs(queries, keys, values, waveforms, gains, window,
                 atom_indices, shifts):
    """Per-batch DRAM images + replicated small tensors."""
    import ml_dtypes

    queries = np.asarray(queries, dtype=np.float32)
    keys = np.asarray(keys, dtype=np.float32)
    values = np.asarray(values, dtype=np.float32)

    bias = _bias_kernel(waveforms, gains, window, atom_indices, shifts)
    shifted = bias - C_SHIFT
    biasb = np.ascontiguousarray(shifted.reshape(KT, 128).T)      # [128, KT]
    sbias = np.ascontiguousarray(
        (SCH_A * shifted + SCH_B0).reshape(KT, 128).T
    ).astype(np.float32)                                          # [128, KT]

    in_maps = []
    for b in range(B):
        vbf = values[b].astype(ml_dtypes.bfloat16)                # [S, D]
        vb = np.ascontiguousarray(
            vbf.reshape(KT, 128, D).transpose(1, 0, 2).reshape(128, KT * D)
        )
        in_maps.append(
            {
                "qT": np.ascontiguousarray(queries[b].T.astype(np.float16)),
                "kT": np.ascontiguousarray(keys[b].T.astype(np.float16)),
                "vb": vb,
                "biasb": biasb,
                "sbias": sbias,
            }
        )
    return in_maps


def kernel(queries, keys, values, waveforms, gains, window, atom_indices,
           shifts):
    in_maps = _host_inputs(
        queries, keys, values, waveforms, gains, window, atom_indices, shifts
    )
    nc = build_nc()
    res = run_bass_kernel_spmd(nc, in_maps, list(range(B)))
    out = np.stack(
        [np.ascontiguousarray(res.results[b]["oT"].T) for b in range(B)]
    )
    return out.astype(np.float32)


# revision 50
# speedup vs baseline: 1.2328x; 1.0083x over previous
"""Trainium2 Bass kernel for nn_AttentionBasisSynthesizer.

out[b] = softmax(Q[b] @ K[b].T + bias) @ V[b], bias[k] built from a tiny
sinusoidal atom bank (computed on host, replicated to every core).

Sharding: data-parallel over the batch dim — 8 batches onto 8 NeuronCores,
one batch per core. Each core computes its full [S, S] attention.

Device-side layout (per core): scores are computed TRANSPOSED, sT[k, q], so
- the key-dim bias is per-partition and folds into the ACT engine's free
  exp(scale*x + bias) affine,
- P @ V needs no transposes: out.T[d, q] = sum over k-tiles of
  matmul(lhsT=V_tile[k,d], rhs=exp_scores[k, q]).
Softmax uses a constant shift C instead of a per-row max (mathematically
exact; scores ~ N(0, sqrt(128)) so exp(s - C) can neither overflow bf16 nor
flush the row maximum for any plausible data).

Structure (engine balance + head/tail overlap):
- The q axis is processed in two halves of 1024. PSUM: four [128, 512]
  PV accumulator banks o_q[h][c] + two [128, 1024] score tiles in
  rotation = exactly 8 banks.
- Per (half, k-tile): QK (2x N=512 fp32r matmuls) -> exp -> PV (2x N=512
  bf16 matmuls). Emission pins each engine's queue order (the Tile
  scheduler is otherwise free-order): QK feeds lead every block so the
  ACT engine — the global bottleneck — never starves.
- exp work is balanced across ACT and DVE: on SPLIT tiles, ACT computes
  q-chunk 0 while the DVE computes q-chunk 1 concurrently via the
  Schraudolph bit-trick, bf16(e^x) ~= bitcast_bf16(uint16(A*x + B_k)),
  one tensor_scalar (mult-imm, add-per-partition-vec) into a separate
  tile (the f32->uint16 saturating convert clamps underflow to +0).
  DVE exps are emitted one block early so they overlap the previous
  tile's ACT exp.
- Z[q] = sum_k p[k,q] runs as two per-chunk bf16 fold chains on the DVE;
  the final 128-partition reduction is a ones-matmul into idle PSUM.
  Half A's lands in half B's accumulators just before B's PVs start
  (B's first PV pairs are deferred past A's reciprocal read); half B's
  lands in half A's long-dead accumulators.
- Half A's tail (reciprocal, normalize, store) overlaps half B's main
  loop; half B's closing tile is cut into 512/384/128 sub-chunks so the
  kernel's last dependence chain (Z -> recip -> mul -> DMA) is short.
- V is converted to bf16 and laid out [k%128, k//128, d] on the host.
"""

import numpy as np

import concourse.bass as bass
import concourse.tile as tile
from concourse import mybir
from concourse.bass_utils import run_bass_kernel_spmd
from concourse.vector_clock import ScopedClock

B, S, D = 8, 2048, 128
KT = S // 128           # 16 key tiles of 128
HW = S // 2             # half width (q) = 1024
C_SHIFT = 20.0          # constant softmax shift (exact: softmax(x-C)=softmax(x))

# Schraudolph exp constants for bf16 (unit in last place of exponent = 128):
# e^x ~= bitcast_bf16(uint16(A*x + B)); B = 127*128 - C0, C0 tuned minimax.
SCH_A = 128.0 / float(np.log(2.0))
SCH_B0 = 128.0 * 127.0 - 5.5

F32 = mybir.dt.float32
F32R = mybir.dt.float32r
F16 = mybir.dt.float16
BF16 = mybir.dt.bfloat16
U16 = mybir.dt.uint16
EXP = mybir.ActivationFunctionType.Exp
MULT = mybir.AluOpType.mult
ADD = mybir.AluOpType.add

# per-half k-tiles whose exp runs entirely on the DVE via the bit-trick,
# with their QK scores staged in the opposite half's idle accumulator
# banks (so they leave the score-tile rotation, and the ACT engine skips
# them without a bubble). Half A tiles must clear before the boundary Z;
# half B tiles must start after half A's normalize frees the banks.
OFF_A = (3, 9)
OFF_B = (5, 9)
WARM = 1                # PE warmup matmul count
FZ = 2                  # trailing p-tiles per half reduced via Z-matmul
                        # instead of the DVE fold chain


def _install_tile_drain_patch():
    """This container's walrus accepts only one semaphore wait per sync-queue
    instruction, but TileContext's tail drain carries one wait per tracked
    proc. Split the waits across single-wait NOPs ahead of the drain (the
    sync queue is in-order, so the drain still begins only after every wait
    has been satisfied)."""

    def _drain_and_barrier(self, tick_clock, wait_clock):
        nc = self.nc
        probe = nc.sync.nop()
        wait_clock.add_sem_waits(
            probe.ins, ScopedClock({None: tick_clock.global_clock})
        )
        si = probe.ins.sync_info
        waits = list(si.on_wait or []) if si is not None else []
        if len(waits) > 1:
            si.on_wait = waits[:1]
            # distribute the remaining waits across all engine queues so the
            # checks evaluate in parallel; the following all-engine barrier
            # joins them back together.
            engines = [nc.sync, nc.scalar, nc.vector, nc.gpsimd, nc.tensor]
            for i, w in enumerate(waits[1:]):
                extra = engines[i % len(engines)].nop()
                extra.ins.sync_info = mybir.SyncInfo(on_wait=[w], on_update=[])
        nc.sync.drain()
        nc.all_engine_barrier()
        assert self.sems is not None
        popped = nc._tile_sem_poison_stack.pop()
        assert popped is self._sem_poison
        nc.clear_and_free_semaphores(list(self.sems.allocated().values()))
        nc.all_engine_barrier()

    tile.TileContext._drain_and_barrier = _drain_and_barrier


def _split_multi_waits(nc: bass.Bass, limit: int = 1) -> int:
    """This container's walrus rejects instructions carrying more than one
    semaphore wait ("Too many sync wait commands"). Hoist excess waits onto
    same-engine NOPs inserted immediately before the instruction — engine
    queues dispatch in order, so the semantics are identical."""
    n_split = 0
    for fn in nc.m.functions:
        for blk in fn.blocks:
            insts = blk.instructions
            out = []
            for inst in insts:
                si = inst.sync_info
                waits = list(si.on_wait or []) if si is not None else []
                if len(waits) > limit:
                    keep = waits[:limit]
                    extra = waits[limit:]
                    for j in range(0, len(extra), limit):
                        nop = mybir.InstNoOp(
                            name=f"{inst.name}-waitsplit{j}",
                            ins=[],
                            outs=[],
                            engine=inst.engine,
                        )
                        nop.sync_info = mybir.SyncInfo(
                            on_wait=extra[j : j + limit], on_update=[]
                        )
                        nc.register_instruction(nop, overwrite=True)
                        out.append(nop)
                        n_split += 1
                    si.on_wait = keep
                out.append(inst)
            if n_split:
                blk.instructions = out
    return n_split


def build_nc(reps: int = 1, warm: int = WARM, off_a=OFF_A, off_b=OFF_B,
             fz: int = FZ) -> bass.Bass:
    """reps>1 unrolls the whole body (including input loads) that many times
    inside one NEFF — used only by the test harness to measure steady-state
    per-execution HW time as a marginal; the graded path uses reps=1."""
    _install_tile_drain_patch()
    offs = (frozenset(off_a), frozenset(off_b))
    nc = bass.Bass()

    qT = nc.declare_dram_parameter("qT", [D, S], F16, isOutput=False)
    kT = nc.declare_dram_parameter("kT", [D, S], F16, isOutput=False)
    qkh = nc.declare_dram_parameter("qkh", [D, 128 + HW], F16, isOutput=False)
    vb = nc.declare_dram_parameter("vb", [128, KT * D], BF16, isOutput=False)
    biasb = nc.declare_dram_parameter("biasb", [128, KT], F32, isOutput=False)
    sbias = nc.declare_dram_parameter("sbias", [128, KT], F32, isOutput=False)
    oT = nc.declare_dram_parameter("oT", [D, S], BF16, isOutput=True)

    with tile.TileContext(nc) as tc:
        with (
            tc.tile_pool(name="const", bufs=1) as const,
            tc.tile_pool(name="pp", bufs=6) as pp,
            tc.tile_pool(name="accp", bufs=3) as accp,
            tc.tile_pool(name="tails", bufs=2) as tails,
            tc.tile_pool(name="sps", bufs=2, space="PSUM") as sps,
            tc.tile_pool(name="ops", bufs=1, space="PSUM") as ops,
        ):
            def _emit_body():
                kTs = const.tile([D, S], F16, tag="kTs")
                qTs = const.tile([D, S], F16, tag="qTs")
                qkh_s = const.tile([D, 128 + HW], F16, tag="qkh")
                bias_s = const.tile([128, KT], F32, tag="bias")
                sbias_s = const.tile([128, KT], F32, tag="sbias")
                ones_s = const.tile([128, 128], BF16, tag="ones")
                vbs = const.tile([128, KT, D], BF16, tag="vbs")
                warm_w = const.tile([128, 128], BF16, tag="warm_w")

                # Per-engine program-order chains. The Tile scheduler orders
                # by its own dependency heuristics; these order-only edges pin
                # each engine's queue to the pipeline order designed here.
                _last = {}

                def chain(key, instr):
                    prev = _last.get(key)
                    if prev is not None:
                        tile.add_dep_helper(
                            instr.ins, prev.ins, sync=False, reason="order"
                        )
                    _last[key] = instr
                    return instr

                chain("dve", nc.vector.memset(warm_w[:], 0.0))
                chain("dve", nc.vector.memset(ones_s[:], 1.0))

                # PE warmup: the cost model's p-state ramp clock starts at
                # the first matmul and never resets, so a single early matmul
                # (during the DMA lead-in) makes every loop matmul full-rate.
                warm_ps = sps.tile([128, HW], F32, tag="sp", name="warm_ps")
                for _w in range(warm):
                    chain("pe", nc.tensor.matmul(
                        warm_ps[:, 0:128], lhsT=warm_w[:], rhs=warm_w[:],
                        start=True, stop=True,
                    ))

                # Input DMAs. HWDGE (sync) queue carries the QK operands in
                # need order; the SWDGE (gpsimd) queue carries bias + V in
                # parallel so the first exp unblocks as early as possible.
                chain("pool", nc.gpsimd.dma_start(bias_s[:], biasb[:]))
                chain("sp", nc.sync.dma_start(qkh_s[:], qkh[:]))
                chain("sp", nc.sync.dma_start(qTs[:, 0:1024], qT[:, 0:1024]))
                chain("pool", nc.gpsimd.dma_start(sbias_s[:], sbias[:]))
                chain("sp", nc.sync.dma_start(kTs[:, 128:512], kT[:, 128:512]))
                chain("pool", nc.gpsimd.dma_start(vbs[:, 0:4, :], vb[:, 0:512]))
                chain("sp", nc.sync.dma_start(kTs[:, 512:2048],
                                              kT[:, 512:2048]))
                chain("pool", nc.gpsimd.dma_start(vbs[:, 4:16, :],
                                                  vb[:, 512:2048]))
                chain("sp", nc.sync.dma_start(qTs[:, 1024:2048],
                                              qT[:, 1024:2048]))

                # four independent 1-bank PSUM accumulators: o_q[h][c] holds
                # the PV accumulation for half h, q-chunk c. Separate tiles
                # keep the dependency streams disjoint (tracking is per-tile).
                o_q = [
                    [
                        ops.tile([128, 512], F32, tag=f"o{h}{c}",
                                 name=f"o{h}{c}")
                        for c in range(2)
                    ]
                    for h in range(2)
                ]

                def mm_qk(h, ki):
                    sp = sps.tile([128, HW], F32, tag="sp", name=f"sp{h}_{ki}")
                    for c in range(2):
                        chain("pe", nc.tensor.matmul(
                            sp[:, c * 512 : (c + 1) * 512],
                            lhsT=(qkh_s[:, 0:128] if ki == 0
                                  else kTs[:, ki * 128 : (ki + 1) * 128]),
                            rhs=(qkh_s[:, 128 + c * 512 : 128 + (c + 1) * 512]
                                 if (h, ki) == (0, 0) else
                                 qTs[:, h * HW + c * 512 : h * HW + (c + 1) * 512]),
                            start=True,
                            stop=True,
                        ))
                    return sp

                p_map = {}

                def make_p(h, ki):
                    if (h, ki) not in p_map:
                        p_map[(h, ki)] = pp.tile([128, HW], BF16, tag="p",
                                                 name=f"p{h}_{ki}")
                    return p_map[(h, ki)]

                def csrc(h, ki, c):
                    # AP of tile (h, ki)'s q-chunk c exp values
                    return make_p(h, ki)[:, c * 512 : (c + 1) * 512]

                sp_tiles = {}

                def emit_off(h, ki):
                    # DVE-offloaded tile: QK scores go into the opposite
                    # half's idle accumulator banks (not the sp rotation),
                    # then one Schraudolph tensor_scalar produces the bf16
                    # exp tile. The ACT engine never sees this tile.
                    scr = o_q[1 - h]
                    for c in range(2):
                        chain("pe", nc.tensor.matmul(
                            scr[c][:],
                            lhsT=(qkh_s[:, 0:128] if ki == 0
                                  else kTs[:, ki * 128 : (ki + 1) * 128]),
                            rhs=qTs[:, h * HW + c * 512 : h * HW + (c + 1) * 512],
                            start=True,
                            stop=True,
                        ))
                    p = make_p(h, ki)
                    for c in range(2):
                        chain("dve", nc.vector.tensor_scalar(
                            p[:, c * 512 : (c + 1) * 512].bitcast(U16),
                            scr[c][:],
                            SCH_A,
                            sbias_s[:, ki : ki + 1],
                            MULT,
                            ADD,
                        ))

                from collections import deque
                pv_q = deque()  # deferred PV chunk emissions (thunks)

                def flush_pv(budget):
                    while pv_q and budget > 0:
                        pv_q.popleft()()
                        budget -= 1

                def mm_pv(h, ki, c0=0, c1=2, defer=False):
                    for c in range(c0, c1):
                        def emit(h=h, ki=ki, c=c):
                            chain("pe", nc.tensor.matmul(
                                o_q[h][c][:],
                                lhsT=vbs[:, ki, :],
                                rhs=csrc(h, ki, c),
                                start=(ki == 0),
                                stop=(ki == KT - 1),
                            ))
                        if defer:
                            pv_q.append(emit)
                        else:
                            emit()

                def mm_z(dst, srcs, start, stop, c0=0, c1=2):
                    # partition-reduce per-chunk sources into dst (a list of
                    # two [128, 512] psum tiles)
                    for c in range(c0, c1):
                        chain("pe", nc.tensor.matmul(
                            dst[c][:],
                            lhsT=ones_s[:],
                            rhs=srcs[c],
                            start=start,
                            stop=stop,
                        ))

                def exp_act(h, ki, sp):
                    # ACT exp: full tile, or chunk 0 only for SPLIT tiles
                    make_p(h, ki)
                    v = p_map[(h, ki)]
                    if isinstance(v, tuple):
                        out, src = v[0][:, 0:512], sp[:, 0:512]
                    else:
                        out, src = v[:, 0:HW], sp[:, 0:HW]
                    chain("act", nc.scalar.activation(
                        out, src, EXP, bias=bias_s[:, ki : ki + 1], scale=1.0,
                    ))

                # Half A folds all 16 p-tiles (its tail overlaps half B's
                # loop, so the fold latency is free) and Z-reduces with one
                # ones-matmul per chunk into half B's accumulators BEFORE
                # B's PV chain starts; B's first PV pairs are deferred past
                # A's reciprocal read and drained a chunk at a time. Half B
                # folds only 0..nfold-1; its last fz p-tiles join Z via
                # PSUM-accumulated ones-matmuls into half A's (long dead)
                # accumulators, so nothing remains after the final exp but
                # one short Z->recip->mul->DMA pipe per closing sub-chunk.
                nfold = KT - fz
                # per-half sequences of ACT-handled tiles; offloaded tiles
                # leave the sp rotation entirely
                seqs = [
                    [ki for ki in range(KT) if ki not in offs[h]]
                    for h in range(2)
                ]
                sp_tiles[(0, seqs[0][0])] = mm_qk(0, seqs[0][0])
                sp_tiles[(0, seqs[0][1])] = mm_qk(0, seqs[0][1])
                pend_a = [None]
                for h in range(2):
                    last = h == 1
                    seq = seqs[h]
                    nf = KT if h == 0 else nfold
                    acc = [None, None]
                    folded = 0  # fold chain caught up through ki < folded
                    rz = tails.tile([128, HW], F32, tag="rz", name=f"rz{h}")
                    oTs = tails.tile([128, HW], BF16, tag="oTs", name=f"oTs{h}")
                    for j, ki in enumerate(seq):
                        closing = last and ki == KT - 1
                        sp = sp_tiles[(h, ki)]
                        # --- exp (the closing tile is chunked so the final
                        # Z->recip->mul->DMA pipeline starts early)
                        if not closing:
                            exp_act(h, ki, sp)
                        else:
                            # sub-order [0:512], [896:1024], [512:896]: all
                            # writes to each o-bank land before any tail
                            # read, so per-tile WAR tracking can't serialize
                            subs = []
                            for a, b, tg in ((0, 512, "clA"),
                                             (896, 1024, "clC"),
                                             (512, 896, "clB")):
                                ps = pp.tile([128, b - a], BF16, tag=tg,
                                             name=f"pcl{a}")
                                chain("act", nc.scalar.activation(
                                    ps[:, 0 : b - a],
                                    sp[:, a:b],
                                    EXP,
                                    bias=bias_s[:, ki : ki + 1],
                                    scale=1.0,
                                ))
                                subs.append((a, b, ps))
                        # --- PE: QK feed first (ACT must never starve), then
                        # offloaded-tile work, then PV work, then Z work
                        if j + 2 < len(seq):
                            nk = seq[j + 2]
                            sp_tiles[(h, nk)] = mm_qk(h, nk)
                        elif not last:
                            nk = seqs[1][j + 2 - len(seq)]
                            sp_tiles[(1, nk)] = mm_qk(1, nk)
                        if ki + 1 in offs[h]:
                            emit_off(h, ki + 1)
                        if closing:
                            zb = o_q[0]  # A's accumulators: long since read
                            flush_pv(99)
                            if nf == KT - 1:
                                mm_z(zb, (acc[0][:], acc[1][:]), True, False)
                            # Z matmuls per sub first (they gate the recips),
                            # then PVs, then the tail reads in bank order
                            for (a, b, ps) in subs:
                                c = a // 512
                                osl = slice(a - c * 512, b - c * 512)
                                w = b - a
                                chain("pe", nc.tensor.matmul(
                                    zb[c][:, osl], lhsT=ones_s[:],
                                    rhs=ps[:, 0:w], start=False, stop=True,
                                ))
                                if a == 0:
                                    chain("dve", nc.vector.reciprocal(
                                        rz[:, 0:512], zb[0][:]))
                                chain("pe", nc.tensor.matmul(
                                    o_q[h][c][:, osl], lhsT=vbs[:, ki, :],
                                    rhs=ps[:, 0:w], start=False, stop=True,
                                ))
                                if a == 0:
                                    # chunk 0 normalizes as soon as its bank
                                    # closes; its store rides the SWDGE queue
                                    chain("dve", nc.vector.tensor_mul(
                                        oTs[:, 0:512], o_q[h][0][:],
                                        rz[:, 0:512],
                                    ))
                                    chain("pool", nc.gpsimd.dma_start(
                                        oT[:, h * HW : h * HW + 512],
                                        oTs[:, 0:512],
                                    ))
                            # bank 1 tail: both sub-ranges' writes are done;
                            # the smallest chunk runs last so the closing
                            # DMA edge is as short as possible
                            for a, b in ((512, 896), (896, 1024)):
                                sl = slice(a, b)
                                osl = slice(a - 512, b - 512)
                                chain("dve", nc.vector.reciprocal(
                                    rz[:, sl], zb[1][:, osl]))
                                chain("dve", nc.vector.tensor_mul(
                                    oTs[:, sl], o_q[h][1][:, osl], rz[:, sl]
                                ))
                                chain("sp", nc.sync.dma_start(
                                    oT[:, h * HW + a : h * HW + b],
                                    oTs[:, sl],
                                ))
                            continue
                        if last and ki == 0 and pend_a[0] is not None:
                            pend_a[0]()  # half A's Z + reciprocal
                        mm_pv(h, ki, defer=last)
                        if ki - 1 in offs[h]:
                            mm_pv(h, ki - 1, defer=last)
                        # B tiles 0-1: emit no PVs (they must trail A's
                        # reciprocal read of B's accumulators); then drain
                        if not (last and ki <= 1):
                            flush_pv(3)
                        if last and ki == nf:
                            mm_z(o_q[0], (acc[0][:], acc[1][:]), True, False)
                        if last and nf <= ki < KT - 1:
                            mm_z(o_q[0], (csrc(h, ki, 0), csrc(h, ki, 1)),
                                 False, False)
                        if last and ki == 2 and pend_a[0] is not None:
                            pend_a[1]()  # half A's normalize + store
                        # --- DVE fold chains (bf16 2x mode), one per
                        # q-chunk, catching up over offloaded tiles
                        while folded <= min(ki, nf - 1):
                            kk = folded
                            if kk == 1:
                                for c in range(2):
                                    acc[c] = accp.tile([128, 512], BF16,
                                                       tag=f"acc{c}",
                                                       name=f"acc{c}_{h}_{kk}")
                                    chain("dve", nc.vector.tensor_add(
                                        acc[c][:], csrc(h, 0, c), csrc(h, 1, c)
                                    ))
                            elif kk >= 2:
                                for c in range(2):
                                    nacc = accp.tile([128, 512], BF16,
                                                     tag=f"acc{c}",
                                                     name=f"acc{c}_{h}_{kk}")
                                    chain("dve", nc.vector.tensor_add(
                                        nacc[:], acc[c][:], csrc(h, kk, c)
                                    ))
                                    acc[c] = nacc
                            folded += 1
                        if h == 0 and ki == KT - 1:
                            acc_a = (acc[0], acc[1])
                            rz_a, oTs_a = rz, oTs

                            def _a_z():
                                # Z for half A: one ones-matmul per chunk
                                # into B's accumulators, then reciprocal;
                                # B's PVs re-init those banks afterwards.
                                mm_z(o_q[1], (acc_a[0][:], acc_a[1][:]),
                                     True, True)
                                sl0, sl1 = slice(0, 512), slice(512, 1024)
                                chain("dve", nc.vector.reciprocal(
                                    rz_a[:, sl0], o_q[1][0][:]))
                                chain("dve", nc.vector.reciprocal(
                                    rz_a[:, sl1], o_q[1][1][:]))

                            def _a_norm():
                                for c in range(2):
                                    sl = slice(c * 512, (c + 1) * 512)
                                    chain("dve", nc.vector.tensor_mul(
                                        oTs_a[:, sl], o_q[0][c][:], rz_a[:, sl]
                                    ))
                                    chain("sp", nc.sync.dma_start(
                                        oT[:, c * 512 : (c + 1) * 512],
                                        oTs_a[:, sl],
                                    ))

                            pend_a[0] = _a_z
                            pend_a.append(_a_norm)

            for _rep in range(reps):
                _emit_body()

    _split_multi_waits(nc)
    return nc


def _bias_kernel(waveforms, gains, window, atom_indices, shifts) -> np.ndarray:
    waveforms = np.asarray(waveforms, dtype=np.float32)
    gains = np.asarray(gains, dtype=np.float32)
    window = np.asarray(window, dtype=np.float32)
    atom_indices = np.asarray(atom_indices).astype(np.int64)
    shifts = np.asarray(shifts).astype(np.int64)
    atoms = waveforms[atom_indices, :S]                  # [P, S]
    bases = atoms * gains[:, None]                       # [P, S]
    shifted = np.stack(
        [np.roll(bases[p], shifts[p]) for p in range(bases.shape[0])]
    )
    return (shifted * window[None, :S]).sum(0).astype(np.float32)  # [S]


def _host_inputs(queries, keys, values, waveforms, gains, window,
                 atom_indices, shifts):
    """Per-batch DRAM images + replicated small tensors."""
    import ml_dtypes

    queries = np.asarray(queries, dtype=np.float32)
    keys = np.asarray(keys, dtype=np.float32)
    values = np.asarray(values, dtype=np.float32)

    bias = _bias_kernel(waveforms, gains, window, atom_indices, shifts)
    shifted = bias - C_SHIFT
    biasb = np.ascontiguousarray(shifted.reshape(KT, 128).T)      # [128, KT]
    sbias = np.ascontiguousarray(
        (SCH_A * shifted + SCH_B0).reshape(KT, 128).T
    ).astype(np.float32)                                          # [128, KT]

    in_maps = []
    for b in range(B):
        vbf = values[b].astype(ml_dtypes.bfloat16)                # [S, D]
        vb = np.ascontiguousarray(
            vbf.reshape(KT, 128, D).transpose(1, 0, 2).reshape(128, KT * D)
        )
        qTb = queries[b].T.astype(np.float16)
        kTb = keys[b].T.astype(np.float16)
        in_maps.append(
            {
                "qT": np.ascontiguousarray(qTb),
                "kT": np.ascontiguousarray(kTb),
                "qkh": np.ascontiguousarray(
                    np.concatenate([kTb[:, 0:128], qTb[:, 0:HW]], axis=1)
                ),
                "vb": vb,
                "biasb": biasb,
                "sbias": sbias,
            }
        )
    return in_maps


def kernel(queries, keys, values, waveforms, gains, window, atom_indices,
           shifts):
    in_maps = _host_inputs(
        queries, keys, values, waveforms, gains, window, atom_indices, shifts
    )
    nc = build_nc()
    res = run_bass_kernel_spmd(nc, in_maps, list(range(B)))
    out = np.stack(
        [np.ascontiguousarray(res.results[b]["oT"].astype(np.float32).T)
         for b in range(B)]
    )
    return out.astype(np.float32)
